# revision 3
# baseline (speedup 1.0000x reference)
"""DiffNet forward pass as a Bass/Tile kernel on 8 Trainium2 NeuronCores. v2.

Data-parallel over the batch (2048 rows/core), tables replicated. The HW
gather primitive is an SWDGE indirect DMA: 128 descriptors (one row per
partition) per ~1.4us instruction — the hard floor is the instruction rate,
so v2 removes instructions from the gather queue and overlap losses:

  - category gathers (16 instrs in v1) are replaced by a one-hot matmul:
    cT[66, 2048] = sum_ch c66_chunk[128, 66]^T @ S_ch[128, 2048], with the
    0/1 selection matrix S built host-side (bf16), then transposed back
    per-slot on the idle PE. Indirect-DMA instructions: 481 -> 465.
  - neighbor gathers fetch 64-wide (elements idx*66..+64 of u66) into
    [128, 8, 64] group tiles; a 3-level DVE tree + acc add replaces 8
    serial adds (fewer DVE instrs and fewer WAR semaphores on the gather
    queue).
  - rows are sorted by neighbor count per core (host permutation), slot
    k-major DESCENDING so the dense phase hides under the gather stream.

Batch row r of a core maps to (partition p, slot t) with r = t*128 + p.
"""

import os
import sys

for _p in ("/opt/trn_rl_repo",):
    if os.path.isdir(_p) and _p not in sys.path:
        sys.path.append(_p)

import numpy as np
import ml_dtypes
from contextlib import ExitStack

import concourse.bass as bass
import concourse.bacc as bacc
import concourse.tile as tile
from concourse import mybir
from concourse.bass_utils import run_bass_kernel_spmd
from concourse.masks import make_identity

N_USERS = 1_000_000
N_PRODUCTS = 500_000
N_CAT = 1_000
F = 64
FA = 66                   # augmented row: emb(64) + 2 bias lanes
L = 2
B = 16384
K = 50

NCORES = 8
BC = B // NCORES          # 2048 rows per core
P = 128                   # partitions
T = BC // P               # 16 slots per partition
CH = 4 * P                # 512 transposed columns per matmul chunk
NCH = BC // CH            # 4 chunks
CCH = 8                   # category one-hot chunks (8 x 128 >= 1000)
G8 = 8                    # neighbor gather group size

dt = mybir.dt


def _build_program(k_sched):
    """k_sched: tuple of T ints — number of neighbor gathers per slot."""
    nc = bacc.Bacc("TRN2", target_bir_lowering=False, debug=False,
                   dynamic_dma_scratch_size=32768)

    NG_N = sum(k_sched)
    NI = 2 * T + NG_N
    # u66 has an appended all-zero row N_USERS: padded neighbor slots point at
    # it, so their contribution to the accumulated sum is exactly zero.
    u66 = nc.dram_tensor("u66", [N_USERS + 1, FA], dt.float32, kind="ExternalInput").ap()
    p66 = nc.dram_tensor("p66", [N_PRODUCTS, FA], dt.float32, kind="ExternalInput").ap()
    c66p = nc.dram_tensor("c66p", [P, CCH, FA], dt.bfloat16, kind="ExternalInput").ap()
    smat = nc.dram_tensor("smat", [P, CCH, BC], dt.bfloat16, kind="ExternalInput").ap()
    wt = nc.dram_tensor("wt", [L, F, F], dt.float32, kind="ExternalInput").ap()
    bv = nc.dram_tensor("bv", [L, F], dt.float32, kind="ExternalInput").ap()
    idxp = nc.dram_tensor("idxp", [P, NI], dt.int32, kind="ExternalInput").ap()
    invc = nc.dram_tensor("invc", [P, T], dt.float32, kind="ExternalInput").ap()
    hnot = nc.dram_tensor("hnot", [1, BC], dt.float32, kind="ExternalInput").ap()
    out_d = nc.dram_tensor("out", [T, P], dt.float32, kind="ExternalOutput").ap()

    f32 = dt.float32
    AX = mybir.AxisListType
    OP = mybir.AluOpType
    AF = mybir.ActivationFunctionType

    with tile.TileContext(nc) as tc, ExitStack() as ctx:
        sp = ctx.enter_context(tc.tile_pool(name="s", bufs=1))
        ptp = ctx.enter_context(tc.tile_pool(name="ptp", bufs=2, space="PSUM"))
        pmm = ctx.enter_context(tc.tile_pool(name="pmm", bufs=2, space="PSUM"))

        # gather index columns first: the gather stream depends only on this
        t_ix = sp.tile([P, NI], dt.int32)
        nc.sync.dma_start(out=t_ix[:, 0:2 * T], in_=idxp[:, 0:2 * T])
        nc.sync.dma_start(out=t_ix[:, 2 * T:], in_=idxp[:, 2 * T:])

        ident = sp.tile([P, P], f32)
        ones1 = sp.tile([1, F], f32)

        # small loads ride the ACT HWDGE queue
        t_invc = sp.tile([P, T], f32)
        nc.scalar.dma_start(out=t_invc[:], in_=invc[:, :])
        t_hnot = sp.tile([1, BC], f32)
        nc.scalar.dma_start(out=t_hnot[:], in_=hnot[:, :])
        t_wt = sp.tile([F, L, F], f32)
        nc.scalar.dma_start(out=t_wt[:], in_=wt.rearrange("l i o -> i l o"))
        t_b = sp.tile([F, L], f32)
        nc.scalar.dma_start(out=t_b[:], in_=bv.rearrange("l f -> f l"))
        t_c66 = sp.tile([P, CCH, FA], dt.bfloat16)
        nc.scalar.dma_start(out=t_c66[:], in_=c66p[:, :, :])
        t_smat = sp.tile([P, CCH, BC], dt.bfloat16)
        nc.scalar.dma_start(out=t_smat[:], in_=smat[:, :, :])

        def gather(out_tile, table, col):
            nc.gpsimd.indirect_dma_start(
                out=out_tile, out_offset=None, in_=table[:],
                in_offset=bass.IndirectOffsetOnAxis(ap=t_ix[:, col:col + 1], axis=0))

        # ---- row gathers: u66 / p66 per slot ---------------------------
        t_u = [sp.tile([P, FA], f32, name=f"tu{t}", tag=f"tu{t}") for t in range(T)]
        t_pc = [sp.tile([P, FA], f32, name=f"tpc{t}", tag=f"tpc{t}") for t in range(T)]
        for t in range(T):
            gather(t_u[t][:], u66, t)
            gather(t_pc[t][:], p66, T + t)

        make_identity(nc, ident[:])
        nc.vector.memset(ones1[:], 1.0)

        # ---- category one-hot matmul: cT[66, BC], then per-slot back-T -
        # (runs on PE/ACT/DVE only — zero gather-queue instructions)
        t_cT = sp.tile([FA, BC], f32)
        for cch in range(NCH):
            cs = slice(cch * CH, (cch + 1) * CH)
            pm = pmm.tile([FA, CH], f32, tag="mm")
            for ch in range(CCH):
                nc.tensor.matmul(pm[:], lhsT=t_c66[:, ch, :],
                                 rhs=t_smat[:, ch, cs],
                                 start=(ch == 0), stop=(ch == CCH - 1))
            nc.vector.tensor_copy(out=t_cT[:, cs], in_=pm[:])
        # back-transpose cT per slot and add into t_pc (row-major)
        for t in range(T):
            ptc = ptp.tile([P, FA], f32, tag="tb")
            nc.tensor.transpose(out=ptc[:], in_=t_cT[:, t * P:(t + 1) * P],
                                identity=ident[:FA, :FA])
            nc.vector.tensor_tensor(out=t_pc[t][:], in0=t_pc[t][:],
                                    in1=ptc[:], op=OP.add)

        # ---- has_nbr==0 mask broadcast to [64, BC] via K=1 matmul -----
        t_m0 = sp.tile([F, BC], dt.uint8)
        for cch in range(NCH):
            pm = pmm.tile([F, CH], f32, tag="mm")
            nc.tensor.matmul(pm[:], lhsT=ones1[:], rhs=t_hnot[:, cch * CH:(cch + 1) * CH],
                             start=True, stop=True)
            nc.vector.tensor_copy(out=t_m0[:, cch * CH:(cch + 1) * CH], in_=pm[:])

        # ---- neighbor gathers: slot-major desc, groups of 8, tree adds -
        t_acc = [sp.tile([P, F], f32, name=f"ta{t}", tag=f"ta{t}") for t in range(T)]
        gnp = ctx.enter_context(tc.tile_pool(name="gn", bufs=14))
        t_nm = sp.tile([P, T, F], f32)
        t_uT = sp.tile([F, BC], f32)
        t_nmT = sp.tile([F, BC], f32)
        t_xT = sp.tile([F, BC], f32)
        t_L1 = sp.tile([F, BC], f32)
        t_L2 = sp.tile([F, BC], f32)
        t_uf = sp.tile([P, T, FA], f32)
        t_pcp = sp.tile([P, T, FA], f32)
        t_int = sp.tile([P, T], f32)

        def finish_slot(t):
            nc.vector.tensor_scalar(
                out=t_nm[:, t, :], in0=t_acc[t][:],
                scalar1=t_invc[:, t:t + 1], scalar2=None, op0=OP.mult)
            cs = slice(t * P, (t + 1) * P)
            pt = ptp.tile([F, P], f32, tag="tp")
            nc.tensor.transpose(out=pt[:], in_=t_u[t][:, 0:F], identity=ident[:])
            nc.scalar.copy(out=t_uT[:, cs], in_=pt[:])
            pt2 = ptp.tile([F, P], f32, tag="tp")
            nc.tensor.transpose(out=pt2[:], in_=t_nm[:, t, :], identity=ident[:])
            nc.scalar.copy(out=t_nmT[:, cs], in_=pt2[:])

        def finish_chunk(cch):
            cs = slice(cch * CH, (cch + 1) * CH)
            src_, dst = t_uT, t_L1
            for l in range(L):
                nc.vector.tensor_tensor(out=t_xT[:, cs], in0=src_[:, cs],
                                        in1=t_nmT[:, cs], op=OP.add)
                pm = pmm.tile([F, CH], f32, tag="mm")
                nc.tensor.matmul(pm[:], lhsT=t_wt[:, l, :], rhs=t_xT[:, cs],
                                 start=True, stop=True)
                nc.scalar.activation(out=dst[:, cs], in_=pm[:], func=AF.Relu,
                                     bias=t_b[:, l:l + 1])
                nc.vector.copy_predicated(out=dst[:, cs], mask=t_m0[:, cs],
                                          data=src_[:, cs])
                src_, dst = dst, t_L2
            for t in range(cch * (T // NCH), (cch + 1) * (T // NCH)):
                pt3 = ptp.tile([P, F], f32, tag="tb")
                nc.tensor.transpose(out=pt3[:], in_=t_L2[:, t * P:(t + 1) * P],
                                    identity=ident[:F, :F])
                nc.scalar.copy(out=t_uf[:, t, 0:F], in_=pt3[:])
                nc.vector.tensor_copy(out=t_uf[:, t, F:FA], in_=t_u[t][:, F:FA])
                nc.vector.tensor_copy(out=t_pcp[:, t, :], in_=t_pc[t][:])
            tsl = slice(cch * (T // NCH), (cch + 1) * (T // NCH))
            nc.vector.tensor_tensor(out=t_pcp[:, tsl, :], in0=t_pcp[:, tsl, :],
                                    in1=t_uf[:, tsl, :], op=OP.mult)
            nc.vector.tensor_reduce(out=t_int[:, tsl], in_=t_pcp[:, tsl, :],
                                    axis=AX.X, op=OP.add)

        def finish_slot_dense(t):
            """Per-slot (128-col) dense chain for the last-emitted chunk:
            keeps the post-gather tail to one slot's worth of work."""
            cs = slice(t * P, (t + 1) * P)
            src_, dst = t_uT, t_L1
            for l in range(L):
                nc.vector.tensor_tensor(out=t_xT[:, cs], in0=src_[:, cs],
                                        in1=t_nmT[:, cs], op=OP.add)
                pm = pmm.tile([F, P], f32, tag="mmf")
                nc.tensor.matmul(pm[:], lhsT=t_wt[:, l, :], rhs=t_xT[:, cs],
                                 start=True, stop=True)
                nc.scalar.activation(out=dst[:, cs], in_=pm[:], func=AF.Relu,
                                     bias=t_b[:, l:l + 1])
                nc.vector.copy_predicated(out=dst[:, cs], mask=t_m0[:, cs],
                                          data=src_[:, cs])
                src_, dst = dst, t_L2
            pt3 = ptp.tile([P, F], f32, tag="tb")
            nc.tensor.transpose(out=pt3[:], in_=t_L2[:, cs],
                                identity=ident[:F, :F])
            nc.scalar.copy(out=t_uf[:, t, 0:F], in_=pt3[:])
            nc.vector.tensor_copy(out=t_uf[:, t, F:FA], in_=t_u[t][:, F:FA])
            nc.vector.tensor_copy(out=t_pcp[:, t, :], in_=t_pc[t][:])
            nc.vector.tensor_tensor(out=t_pcp[:, t, :], in0=t_pcp[:, t, :],
                                    in1=t_uf[:, t, :], op=OP.mult)
            nc.vector.tensor_reduce(out=t_int[:, t:t + 1], in_=t_pcp[:, t, :],
                                    axis=AX.X, op=OP.add)

        colbase = []
        c = 2 * T
        for t in range(T):
            colbase.append(c)
            c += k_sched[t]
        done = [False] * T
        TSC = T // NCH

        def slot_done(t):
            done[t] = True
            finish_slot(t)
            if t < TSC:
                # last-emitted chunk: per-slot dense, no chunk barrier
                finish_slot_dense(t)
                return
            cch = t // TSC
            if all(done[cch * TSC:(cch + 1) * TSC]):
                finish_chunk(cch)

        for t in range(T):
            if k_sched[t] == 0:
                nc.vector.memset(t_acc[t][:], 0.0)
                slot_done(t)

        def tree_fold(g8, w):
            """fold first w columns of group tile g8 [P, G8, F] to column 0."""
            while w > 1:
                if w % 2 == 1:
                    nc.vector.tensor_tensor(
                        out=g8[:, 0, :], in0=g8[:, 0, :],
                        in1=g8[:, w - 1, :], op=OP.add)
                    w -= 1
                h = w // 2
                nc.vector.tensor_tensor(
                    out=g8[:, 0:h, :], in0=g8[:, 0:h, :],
                    in1=g8[:, h:2 * h, :], op=OP.add)
                w = h

        # slot-major, DESCENDING size: big slots (and their chunk's dense
        # phase) complete early under the gather stream.
        gi = 0
        for t in range(T - 1, -1, -1):
            kk = k_sched[t]
            if kk == 0:
                continue
            first = True
            for g0 in range(0, kk, G8):
                gn = min(G8, kk - g0)
                g8 = gnp.tile([P, G8, F], f32, name=f"g{gi}", tag="g")
                gi += 1
                for j in range(gn):
                    # 64-wide fetch: elements idx*66 .. idx*66+64 of u66
                    gather(g8[:, j, :], u66, colbase[t] + g0 + j)
                tree_fold(g8, gn)
                if first:
                    nc.vector.tensor_copy(out=t_acc[t][:], in_=g8[:, 0, :])
                    first = False
                else:
                    nc.vector.tensor_tensor(out=t_acc[t][:], in0=t_acc[t][:],
                                            in1=g8[:, 0, :], op=OP.add)
            slot_done(t)

        # ---- output (dot computed per chunk in finish_chunk) ----------
        nc.sync.dma_start(out=out_d.rearrange("t p -> p t"), in_=t_int[:])

    nc.compile()
    return nc


_PROGRAM_CACHE = {}


def _get_program(k_sched):
    key = tuple(k_sched)
    if key not in _PROGRAM_CACHE:
        _PROGRAM_CACHE[key] = _build_program(key)
    return _PROGRAM_CACHE[key]


def kernel(user_idx, product_idx, category_idx, neighbor_idx, neighbor_lens,
           user_emb, product_emb, category_emb, user_bias_tab, product_bias_tab,
           global_bias, W, b, _run_kwargs=None, _return_res=False):
    user_idx = np.asarray(user_idx).astype(np.int32)
    product_idx = np.asarray(product_idx).astype(np.int32)
    category_idx = np.asarray(category_idx).astype(np.int32)
    neighbor_idx = np.asarray(neighbor_idx).astype(np.int32)
    neighbor_lens = np.asarray(neighbor_lens).astype(np.int64)
    user_emb = np.asarray(user_emb, dtype=np.float32)
    product_emb = np.asarray(product_emb, dtype=np.float32)
    category_emb = np.asarray(category_emb, dtype=np.float32)
    user_bias_tab = np.asarray(user_bias_tab, dtype=np.float32)
    product_bias_tab = np.asarray(product_bias_tab, dtype=np.float32)
    gb = float(np.asarray(global_bias, dtype=np.float32))
    W = np.asarray(W, dtype=np.float32)
    b = np.asarray(b, dtype=np.float32)

    # augmented tables: score = dot66(u66_final, p66 + cT)
    u66_t = np.zeros((N_USERS + 1, FA), np.float32)
    u66_t[:N_USERS, :F] = user_emb
    u66_t[:N_USERS, F] = user_bias_tab + gb
    u66_t[:N_USERS, F + 1] = 1.0
    p66_t = np.empty((N_PRODUCTS, FA), np.float32)
    p66_t[:, :F] = product_emb
    p66_t[:, F] = 1.0
    p66_t[:, F + 1] = product_bias_tab
    # category chunks for the one-hot matmul: c66p[p, ch, :] = 0.3*cat[ch*128+p]
    c66p_t = np.zeros((P, CCH, FA), ml_dtypes.bfloat16)
    cat_pad = np.zeros((P * CCH, FA), np.float32)
    cat_pad[:N_CAT, :F] = 0.3 * category_emb
    c66p_t[:, :, :] = cat_pad.reshape(CCH, P, FA).transpose(1, 0, 2).astype(
        ml_dtypes.bfloat16)

    lens = np.clip(neighbor_lens, 0, K).astype(np.int64)

    # per-core sort by neighbor count; schedule shared across cores
    perms, kslots = [], np.zeros((NCORES, T), np.int64)
    for c in range(NCORES):
        lc = lens[c * BC:(c + 1) * BC]
        perm = np.argsort(lc, kind="stable")
        perms.append(perm)
        ls = lc[perm]
        kslots[c] = ls.reshape(T, P).max(axis=1)
    k_sched = tuple(int(x) for x in kslots.max(axis=0))
    nc = _get_program(k_sched)

    in_maps = []
    for c in range(NCORES):
        sl = slice(c * BC, (c + 1) * BC)
        perm = perms[c]
        ui = user_idx[sl][perm]
        pi = product_idx[sl][perm]
        ci = category_idx[sl][perm]
        ni = neighbor_idx[sl][perm]          # [BC, K]
        lc = lens[sl][perm]

        cols = [ui.reshape(T, P), pi.reshape(T, P)]
        ncols = []
        ni3 = ni.reshape(T, P, K)
        lc2 = lc.reshape(T, P)
        for t in range(T):
            kk = k_sched[t]
            col = np.where(np.arange(kk)[None, :] < lc2[t][:, None],
                           ni3[t, :, :kk], N_USERS).astype(np.int32)  # [P, kk]
            ncols.append(col.T)              # [kk, P]
        idxp_np = np.concatenate([np.concatenate(cols, 0).astype(np.int32)]
                                 + ncols, axis=0).T

        # one-hot selection matrix for the category matmul (bf16 0/1)
        smat_np = np.zeros((P, CCH, BC), ml_dtypes.bfloat16)
        colr = np.arange(BC)
        smat_np[ci % P, ci // P, colr] = 1.0

        invc_np = (1.0 / np.maximum(lc2, 1)).astype(np.float32).T.copy()  # [P,T]
        hnot_np = (lc == 0).astype(np.float32).reshape(1, BC)

        in_maps.append({
            "u66": u66_t, "p66": p66_t, "c66p": c66p_t,
            "smat": smat_np,
            "wt": np.ascontiguousarray(W.transpose(0, 2, 1)),
            "bv": np.ascontiguousarray(b),
            "idxp": np.ascontiguousarray(idxp_np),
            "invc": np.ascontiguousarray(invc_np),
            "hnot": hnot_np,
        })

    res = run_bass_kernel_spmd(nc, in_maps, list(range(NCORES)),
                               **(_run_kwargs or {}))
    out = np.empty(B, np.float32)
    for c in range(NCORES):
        o = res.results[c]["out"].reshape(-1)   # sorted order, r = t*128+p
        dst = out[c * BC:(c + 1) * BC]
        dst[perms[c]] = o
    if _return_res:
        return out, res
    return out


# revision 4
# speedup vs baseline: 1.0032x; 1.0032x over previous
"""DiffNet forward pass as a Bass/Tile kernel on 8 Trainium2 NeuronCores. v2.

Data-parallel over the batch (2048 rows/core), tables replicated. The HW
gather primitive is an SWDGE indirect DMA: 128 descriptors (one row per
partition) per ~1.4us instruction — the hard floor is the instruction rate,
so v2 removes instructions from the gather queue and overlap losses:

  - category gathers (16 instrs in v1) are replaced by a one-hot matmul:
    cT[66, 2048] = sum_ch c66_chunk[128, 66]^T @ S_ch[128, 2048], with the
    0/1 selection matrix S built host-side (bf16), then transposed back
    per-slot on the idle PE. Indirect-DMA instructions: 481 -> 465.
  - neighbor gathers fetch 64-wide (elements idx*66..+64 of u66) into
    [128, 8, 64] group tiles; a 3-level DVE tree + acc add replaces 8
    serial adds (fewer DVE instrs and fewer WAR semaphores on the gather
    queue).
  - rows are sorted by neighbor count per core (host permutation), slot
    k-major DESCENDING so the dense phase hides under the gather stream.

Batch row r of a core maps to (partition p, slot t) with r = t*128 + p.
"""

import os
import sys

for _p in ("/opt/trn_rl_repo",):
    if os.path.isdir(_p) and _p not in sys.path:
        sys.path.append(_p)

import numpy as np
import ml_dtypes
from contextlib import ExitStack

import concourse.bass as bass
import concourse.bacc as bacc
import concourse.tile as tile
from concourse import mybir
from concourse.bass_utils import run_bass_kernel_spmd
from concourse.masks import make_identity

N_USERS = 1_000_000
N_PRODUCTS = 500_000
N_CAT = 1_000
F = 64
FA = 66                   # augmented row: emb(64) + 2 bias lanes
L = 2
B = 16384
K = 50

NCORES = 8
BC = B // NCORES          # 2048 rows per core
P = 128                   # partitions
T = BC // P               # 16 slots per partition
CH = 4 * P                # 512 transposed columns per matmul chunk
NCH = BC // CH            # 4 chunks
CCH = 8                   # category one-hot chunks (8 x 128 >= 1000)
G8 = 8                    # neighbor gather group size

dt = mybir.dt


def _build_program(k_sched):
    """k_sched: tuple of T ints — number of neighbor gathers per slot."""
    nc = bacc.Bacc("TRN2", target_bir_lowering=False, debug=False,
                   dynamic_dma_scratch_size=32768)

    NG_N = sum(k_sched)
    NI = 2 * T + NG_N
    # u66 has an appended all-zero row N_USERS: padded neighbor slots point at
    # it, so their contribution to the accumulated sum is exactly zero.
    u66 = nc.dram_tensor("u66", [N_USERS + 1, FA], dt.float32, kind="ExternalInput").ap()
    p66 = nc.dram_tensor("p66", [N_PRODUCTS, FA], dt.float32, kind="ExternalInput").ap()
    c66p = nc.dram_tensor("c66p", [P, CCH, FA], dt.bfloat16, kind="ExternalInput").ap()
    smat = nc.dram_tensor("smat", [P, CCH, BC], dt.bfloat16, kind="ExternalInput").ap()
    wt = nc.dram_tensor("wt", [L, F, F], dt.float32, kind="ExternalInput").ap()
    bv = nc.dram_tensor("bv", [L, F], dt.float32, kind="ExternalInput").ap()
    idxp = nc.dram_tensor("idxp", [P, NI], dt.int32, kind="ExternalInput").ap()
    invc = nc.dram_tensor("invc", [P, T], dt.float32, kind="ExternalInput").ap()
    hnot = nc.dram_tensor("hnot", [1, BC], dt.float32, kind="ExternalInput").ap()
    out_d = nc.dram_tensor("out", [T, P], dt.float32, kind="ExternalOutput").ap()

    f32 = dt.float32
    AX = mybir.AxisListType
    OP = mybir.AluOpType
    AF = mybir.ActivationFunctionType

    with tile.TileContext(nc) as tc, ExitStack() as ctx:
        sp = ctx.enter_context(tc.tile_pool(name="s", bufs=1))
        ptp = ctx.enter_context(tc.tile_pool(name="ptp", bufs=2, space="PSUM"))
        pmm = ctx.enter_context(tc.tile_pool(name="pmm", bufs=2, space="PSUM"))

        # gather index columns first: the gather stream depends only on this
        t_ix = sp.tile([P, NI], dt.int32)
        # u/p index columns ride the gather queue itself: it comes out of the
        # NEFF preamble earliest and the first gather then waits on a
        # same-queue completion instead of a cross-engine semaphore hop
        nc.gpsimd.dma_start(out=t_ix[:, 0:2 * T], in_=idxp[:, 0:2 * T])
        nc.sync.dma_start(out=t_ix[:, 2 * T:], in_=idxp[:, 2 * T:])

        ident = sp.tile([P, P], f32)
        ones1 = sp.tile([1, F], f32)

        # small loads ride the ACT HWDGE queue
        t_invc = sp.tile([P, T], f32)
        nc.scalar.dma_start(out=t_invc[:], in_=invc[:, :])
        t_hnot = sp.tile([1, BC], f32)
        nc.scalar.dma_start(out=t_hnot[:], in_=hnot[:, :])
        t_wt = sp.tile([F, L, F], f32)
        nc.scalar.dma_start(out=t_wt[:], in_=wt.rearrange("l i o -> i l o"))
        t_b = sp.tile([F, L], f32)
        nc.scalar.dma_start(out=t_b[:], in_=bv.rearrange("l f -> f l"))
        t_c66 = sp.tile([P, CCH, FA], dt.bfloat16)
        nc.scalar.dma_start(out=t_c66[:], in_=c66p[:, :, :])
        t_smat = sp.tile([P, CCH, BC], dt.bfloat16)
        nc.scalar.dma_start(out=t_smat[:], in_=smat[:, :, :])

        def gather(out_tile, table, col):
            nc.gpsimd.indirect_dma_start(
                out=out_tile, out_offset=None, in_=table[:],
                in_offset=bass.IndirectOffsetOnAxis(ap=t_ix[:, col:col + 1], axis=0))

        # ---- row gathers: u66 / p66 per slot ---------------------------
        t_u = [sp.tile([P, FA], f32, name=f"tu{t}", tag=f"tu{t}") for t in range(T)]
        t_pc = [sp.tile([P, FA], f32, name=f"tpc{t}", tag=f"tpc{t}") for t in range(T)]
        for t in range(T):
            gather(t_u[t][:], u66, t)
            gather(t_pc[t][:], p66, T + t)

        make_identity(nc, ident[:])
        nc.vector.memset(ones1[:], 1.0)

        # ---- category one-hot matmul: cT[66, BC], then per-slot back-T -
        # (runs on PE/ACT/DVE only — zero gather-queue instructions)
        t_cT = sp.tile([FA, BC], f32)
        for cch in range(NCH):
            cs = slice(cch * CH, (cch + 1) * CH)
            pm = pmm.tile([FA, CH], f32, tag="mm")
            for ch in range(CCH):
                nc.tensor.matmul(pm[:], lhsT=t_c66[:, ch, :],
                                 rhs=t_smat[:, ch, cs],
                                 start=(ch == 0), stop=(ch == CCH - 1))
            nc.vector.tensor_copy(out=t_cT[:, cs], in_=pm[:])
        # back-transpose cT per slot and add into t_pc (row-major)
        for t in range(T):
            ptc = ptp.tile([P, FA], f32, tag="tb")
            nc.tensor.transpose(out=ptc[:], in_=t_cT[:, t * P:(t + 1) * P],
                                identity=ident[:FA, :FA])
            nc.vector.tensor_tensor(out=t_pc[t][:], in0=t_pc[t][:],
                                    in1=ptc[:], op=OP.add)

        # ---- has_nbr==0 mask broadcast to [64, BC] via K=1 matmul -----
        t_m0 = sp.tile([F, BC], dt.uint8)
        for cch in range(NCH):
            pm = pmm.tile([F, CH], f32, tag="mm")
            nc.tensor.matmul(pm[:], lhsT=ones1[:], rhs=t_hnot[:, cch * CH:(cch + 1) * CH],
                             start=True, stop=True)
            nc.vector.tensor_copy(out=t_m0[:, cch * CH:(cch + 1) * CH], in_=pm[:])

        # ---- neighbor gathers: slot-major desc, groups of 8, tree adds -
        t_acc = [sp.tile([P, F], f32, name=f"ta{t}", tag=f"ta{t}") for t in range(T)]
        gnp = ctx.enter_context(tc.tile_pool(name="gn", bufs=14))
        t_nm = sp.tile([P, T, F], f32)
        t_uT = sp.tile([F, BC], f32)
        t_nmT = sp.tile([F, BC], f32)
        t_xT = sp.tile([F, BC], f32)
        t_L1 = sp.tile([F, BC], f32)
        t_L2 = sp.tile([F, BC], f32)
        t_uf = sp.tile([P, T, FA], f32)
        t_pcp = sp.tile([P, T, FA], f32)
        t_int = sp.tile([P, T], f32)

        def finish_slot(t):
            nc.vector.tensor_scalar(
                out=t_nm[:, t, :], in0=t_acc[t][:],
                scalar1=t_invc[:, t:t + 1], scalar2=None, op0=OP.mult)
            cs = slice(t * P, (t + 1) * P)
            pt = ptp.tile([F, P], f32, tag="tp")
            nc.tensor.transpose(out=pt[:], in_=t_u[t][:, 0:F], identity=ident[:])
            nc.scalar.copy(out=t_uT[:, cs], in_=pt[:])
            pt2 = ptp.tile([F, P], f32, tag="tp")
            nc.tensor.transpose(out=pt2[:], in_=t_nm[:, t, :], identity=ident[:])
            nc.scalar.copy(out=t_nmT[:, cs], in_=pt2[:])

        def finish_chunk(cch):
            cs = slice(cch * CH, (cch + 1) * CH)
            src_, dst = t_uT, t_L1
            for l in range(L):
                nc.vector.tensor_tensor(out=t_xT[:, cs], in0=src_[:, cs],
                                        in1=t_nmT[:, cs], op=OP.add)
                pm = pmm.tile([F, CH], f32, tag="mm")
                nc.tensor.matmul(pm[:], lhsT=t_wt[:, l, :], rhs=t_xT[:, cs],
                                 start=True, stop=True)
                nc.scalar.activation(out=dst[:, cs], in_=pm[:], func=AF.Relu,
                                     bias=t_b[:, l:l + 1])
                nc.vector.copy_predicated(out=dst[:, cs], mask=t_m0[:, cs],
                                          data=src_[:, cs])
                src_, dst = dst, t_L2
            for t in range(cch * (T // NCH), (cch + 1) * (T // NCH)):
                pt3 = ptp.tile([P, F], f32, tag="tb")
                nc.tensor.transpose(out=pt3[:], in_=t_L2[:, t * P:(t + 1) * P],
                                    identity=ident[:F, :F])
                nc.scalar.copy(out=t_uf[:, t, 0:F], in_=pt3[:])
                nc.vector.tensor_copy(out=t_uf[:, t, F:FA], in_=t_u[t][:, F:FA])
                nc.vector.tensor_copy(out=t_pcp[:, t, :], in_=t_pc[t][:])
            tsl = slice(cch * (T // NCH), (cch + 1) * (T // NCH))
            nc.vector.tensor_tensor(out=t_pcp[:, tsl, :], in0=t_pcp[:, tsl, :],
                                    in1=t_uf[:, tsl, :], op=OP.mult)
            nc.vector.tensor_reduce(out=t_int[:, tsl], in_=t_pcp[:, tsl, :],
                                    axis=AX.X, op=OP.add)

        def finish_slot_dense(t):
            """Per-slot (128-col) dense chain for the last-emitted chunk:
            keeps the post-gather tail to one slot's worth of work."""
            cs = slice(t * P, (t + 1) * P)
            src_, dst = t_uT, t_L1
            for l in range(L):
                nc.vector.tensor_tensor(out=t_xT[:, cs], in0=src_[:, cs],
                                        in1=t_nmT[:, cs], op=OP.add)
                pm = pmm.tile([F, P], f32, tag="mmf")
                nc.tensor.matmul(pm[:], lhsT=t_wt[:, l, :], rhs=t_xT[:, cs],
                                 start=True, stop=True)
                nc.scalar.activation(out=dst[:, cs], in_=pm[:], func=AF.Relu,
                                     bias=t_b[:, l:l + 1])
                nc.vector.copy_predicated(out=dst[:, cs], mask=t_m0[:, cs],
                                          data=src_[:, cs])
                src_, dst = dst, t_L2
            pt3 = ptp.tile([P, F], f32, tag="tb")
            nc.tensor.transpose(out=pt3[:], in_=t_L2[:, cs],
                                identity=ident[:F, :F])
            nc.scalar.copy(out=t_uf[:, t, 0:F], in_=pt3[:])
            nc.vector.tensor_copy(out=t_uf[:, t, F:FA], in_=t_u[t][:, F:FA])
            nc.vector.tensor_copy(out=t_pcp[:, t, :], in_=t_pc[t][:])
            nc.vector.tensor_tensor(out=t_pcp[:, t, :], in0=t_pcp[:, t, :],
                                    in1=t_uf[:, t, :], op=OP.mult)
            nc.vector.tensor_reduce(out=t_int[:, t:t + 1], in_=t_pcp[:, t, :],
                                    axis=AX.X, op=OP.add)

        colbase = []
        c = 2 * T
        for t in range(T):
            colbase.append(c)
            c += k_sched[t]
        done = [False] * T
        TSC = T // NCH

        def slot_done(t):
            done[t] = True
            finish_slot(t)
            if t < TSC:
                # last-emitted chunk: per-slot dense, no chunk barrier
                finish_slot_dense(t)
                return
            cch = t // TSC
            if all(done[cch * TSC:(cch + 1) * TSC]):
                finish_chunk(cch)

        for t in range(T):
            if k_sched[t] == 0:
                nc.vector.memset(t_acc[t][:], 0.0)
                slot_done(t)

        def tree_fold(g8, w):
            """fold first w columns of group tile g8 [P, G8, F] to column 0."""
            while w > 1:
                if w % 2 == 1:
                    nc.vector.tensor_tensor(
                        out=g8[:, 0, :], in0=g8[:, 0, :],
                        in1=g8[:, w - 1, :], op=OP.add)
                    w -= 1
                h = w // 2
                nc.vector.tensor_tensor(
                    out=g8[:, 0:h, :], in0=g8[:, 0:h, :],
                    in1=g8[:, h:2 * h, :], op=OP.add)
                w = h

        # slot-major, DESCENDING size: big slots (and their chunk's dense
        # phase) complete early under the gather stream.
        gi = 0
        for t in range(T - 1, -1, -1):
            kk = k_sched[t]
            if kk == 0:
                continue
            first = True
            for g0 in range(0, kk, G8):
                gn = min(G8, kk - g0)
                g8 = gnp.tile([P, G8, F], f32, name=f"g{gi}", tag="g")
                gi += 1
                for j in range(gn):
                    # 64-wide fetch: elements idx*66 .. idx*66+64 of u66
                    gather(g8[:, j, :], u66, colbase[t] + g0 + j)
                tree_fold(g8, gn)
                if first:
                    nc.vector.tensor_copy(out=t_acc[t][:], in_=g8[:, 0, :])
                    first = False
                else:
                    nc.vector.tensor_tensor(out=t_acc[t][:], in0=t_acc[t][:],
                                            in1=g8[:, 0, :], op=OP.add)
            slot_done(t)

        # ---- output (dot computed per chunk in finish_chunk) ----------
        nc.sync.dma_start(out=out_d.rearrange("t p -> p t"), in_=t_int[:])

    nc.compile()
    return nc


_PROGRAM_CACHE = {}


def _get_program(k_sched):
    key = tuple(k_sched)
    if key not in _PROGRAM_CACHE:
        _PROGRAM_CACHE[key] = _build_program(key)
    return _PROGRAM_CACHE[key]


def kernel(user_idx, product_idx, category_idx, neighbor_idx, neighbor_lens,
           user_emb, product_emb, category_emb, user_bias_tab, product_bias_tab,
           global_bias, W, b, _run_kwargs=None, _return_res=False):
    user_idx = np.asarray(user_idx).astype(np.int32)
    product_idx = np.asarray(product_idx).astype(np.int32)
    category_idx = np.asarray(category_idx).astype(np.int32)
    neighbor_idx = np.asarray(neighbor_idx).astype(np.int32)
    neighbor_lens = np.asarray(neighbor_lens).astype(np.int64)
    user_emb = np.asarray(user_emb, dtype=np.float32)
    product_emb = np.asarray(product_emb, dtype=np.float32)
    category_emb = np.asarray(category_emb, dtype=np.float32)
    user_bias_tab = np.asarray(user_bias_tab, dtype=np.float32)
    product_bias_tab = np.asarray(product_bias_tab, dtype=np.float32)
    gb = float(np.asarray(global_bias, dtype=np.float32))
    W = np.asarray(W, dtype=np.float32)
    b = np.asarray(b, dtype=np.float32)

    # augmented tables: score = dot66(u66_final, p66 + cT)
    u66_t = np.zeros((N_USERS + 1, FA), np.float32)
    u66_t[:N_USERS, :F] = user_emb
    u66_t[:N_USERS, F] = user_bias_tab + gb
    u66_t[:N_USERS, F + 1] = 1.0
    p66_t = np.empty((N_PRODUCTS, FA), np.float32)
    p66_t[:, :F] = product_emb
    p66_t[:, F] = 1.0
    p66_t[:, F + 1] = product_bias_tab
    # category chunks for the one-hot matmul: c66p[p, ch, :] = 0.3*cat[ch*128+p]
    c66p_t = np.zeros((P, CCH, FA), ml_dtypes.bfloat16)
    cat_pad = np.zeros((P * CCH, FA), np.float32)
    cat_pad[:N_CAT, :F] = 0.3 * category_emb
    c66p_t[:, :, :] = cat_pad.reshape(CCH, P, FA).transpose(1, 0, 2).astype(
        ml_dtypes.bfloat16)

    lens = np.clip(neighbor_lens, 0, K).astype(np.int64)

    # per-core sort by neighbor count; schedule shared across cores
    perms, kslots = [], np.zeros((NCORES, T), np.int64)
    for c in range(NCORES):
        lc = lens[c * BC:(c + 1) * BC]
        perm = np.argsort(lc, kind="stable")
        perms.append(perm)
        ls = lc[perm]
        kslots[c] = ls.reshape(T, P).max(axis=1)
    k_sched = tuple(int(x) for x in kslots.max(axis=0))
    nc = _get_program(k_sched)

    in_maps = []
    for c in range(NCORES):
        sl = slice(c * BC, (c + 1) * BC)
        perm = perms[c]
        ui = user_idx[sl][perm]
        pi = product_idx[sl][perm]
        ci = category_idx[sl][perm]
        ni = neighbor_idx[sl][perm]          # [BC, K]
        lc = lens[sl][perm]

        cols = [ui.reshape(T, P), pi.reshape(T, P)]
        ncols = []
        ni3 = ni.reshape(T, P, K)
        lc2 = lc.reshape(T, P)
        for t in range(T):
            kk = k_sched[t]
            col = np.where(np.arange(kk)[None, :] < lc2[t][:, None],
                           ni3[t, :, :kk], N_USERS).astype(np.int32)  # [P, kk]
            ncols.append(col.T)              # [kk, P]
        idxp_np = np.concatenate([np.concatenate(cols, 0).astype(np.int32)]
                                 + ncols, axis=0).T

        # one-hot selection matrix for the category matmul (bf16 0/1)
        smat_np = np.zeros((P, CCH, BC), ml_dtypes.bfloat16)
        colr = np.arange(BC)
        smat_np[ci % P, ci // P, colr] = 1.0

        invc_np = (1.0 / np.maximum(lc2, 1)).astype(np.float32).T.copy()  # [P,T]
        hnot_np = (lc == 0).astype(np.float32).reshape(1, BC)

        in_maps.append({
            "u66": u66_t, "p66": p66_t, "c66p": c66p_t,
            "smat": smat_np,
            "wt": np.ascontiguousarray(W.transpose(0, 2, 1)),
            "bv": np.ascontiguousarray(b),
            "idxp": np.ascontiguousarray(idxp_np),
            "invc": np.ascontiguousarray(invc_np),
            "hnot": hnot_np,
        })

    res = run_bass_kernel_spmd(nc, in_maps, list(range(NCORES)),
                               **(_run_kwargs or {}))
    out = np.empty(B, np.float32)
    for c in range(NCORES):
        o = res.results[c]["out"].reshape(-1)   # sorted order, r = t*128+p
        dst = out[c * BC:(c + 1) * BC]
        dst[perms[c]] = o
    if _return_res:
        return out, res
    return out
